# revision 31
# baseline (speedup 1.0000x reference)
"""Trainium2 Bass kernel for nn_MistralMoLoraLayer (MoE-routed LoRA FFN).

Strategy: data-parallel over tokens (8 cores x 256 tokens), base FFN weights
replicated in bf16, all-expert LoRA replicated in bf16. The per-(batch,slot)
softmax over the sequence axis needs global denominators -> tiny [2,8]
AllReduce, hidden behind a W-deep base-GEMM software pipeline.

Per-core phases (tiles are [128 partitions, tokens free]):
  router: exact-f32 logits from bf16 pairs (xb+xr)@(gwb+gwr); top-2
          (value,index) per token; exp; AllReduce of per-batch-slot sums;
          weights w_j = exp_j/denom. Emitted in small stages between warmup
          base tiles; only cb (weights) waits on the AR.
  A-proj: UA/GA [E*R=128, t] = stacked up_A/gate_A @ x.T (bf16, K=D chain)
  slot-mask trick: Ut_j = UA * M_j with M_j[e*R+r,t] = (sel_j(t)==e) so a
          single [128,128] B matmul applies the routed expert's LoRA-B
  h-loop: software-pipelined; base GEMM leads the lora stage by W tiles to
          hide router+AR latency, then double-lora iterations shrink the
          lag to 3 so there is no tail drain.
      base: psUG = [w_up|w_gate] chain @ x  -> U/G ring (bf16, Act copies)
      lora: psLOu/g = up_B/gate_B @ Ut/Gt (double-buffered banks);
            h_j = silu(U+lo_u)*(G+lo_g) with slot0 via Act copies and
            slot1 via direct DVE PSUM reads (engine balance);
            ch_j = c_j*h_j; mixed = ch_0+ch_1; psV += down_A @ ch (lag 2)
  down:   outT = w_down chain @ mixed + down_B @ (v_0+v_1); wd streamed in
          quarters, 5 d-tiles prefetched during the h-loop
"""

import numpy as np

# problem constants (hardcoded; kernel.py must be self-contained)
B, S, D, H, E, R, TOPK = 2, 1024, 2048, 5632, 8, 16, 2
ALPHA = 2.0
T = B * S
NCORES = 8
TC = T // NCORES           # 256 tokens per core
KT = D // 128              # 16 k-tiles over D
HT = H // 128              # 44 h-tiles
DT = D // 128              # 16 d-tiles
ER = E * R                 # 128

W = 12                     # base-GEMM lead (pipeline depth, h-tiles)
NPRE = 5                   # wd d-tiles prefetched during h-loop
DEBUG_TAPS = False
SKIP_AR = False            # replace AllReduce with local copy (TimelineSim)

_cache = {}


def _build():
    import concourse.bacc as bacc
    import concourse.bass as bass
    import concourse.mybir as mybir
    import concourse.tile as tile
    from concourse.masks import make_identity

    f32 = mybir.dt.float32
    bf16 = mybir.dt.bfloat16
    AL = mybir.AluOpType
    AF = mybir.ActivationFunctionType

    nc = bacc.Bacc("TRN2", target_bir_lowering=False, debug=False,
                   num_devices=NCORES)

    # ---- DRAM I/O ----
    d_xTb = nc.dram_tensor("xTb", [128, KT * TC], bf16,
                           kind="ExternalInput").ap()
    d_xr = nc.dram_tensor("xr", [128, KT * TC], bf16,
                          kind="ExternalInput").ap()
    d_gwb = nc.dram_tensor("gwb", [128, KT * E], bf16,
                           kind="ExternalInput").ap()
    d_gwr = nc.dram_tensor("gwr", [128, KT * E], bf16,
                           kind="ExternalInput").ap()
    d_wug = nc.dram_tensor("wug", [HT, 128, 2 * KT * 128], bf16,
                           kind="ExternalInput").ap()
    d_wd = nc.dram_tensor("wd", [DT, 128, HT * 128], bf16,
                          kind="ExternalInput").ap()
    d_A = nc.dram_tensor("Ah", [128, KT * 2 * ER], bf16,
                         kind="ExternalInput").ap()
    d_lora = nc.dram_tensor("lorah", [HT, 128, 3 * 128], bf16,
                            kind="ExternalInput").ap()
    d_dB = nc.dram_tensor("dB", [128, D], bf16, kind="ExternalInput").ap()
    d_eid = nc.dram_tensor("eid", [128, 1], f32, kind="ExternalInput").ap()
    d_i8m = nc.dram_tensor("i8m", [128, E], f32, kind="ExternalInput").ap()
    d_bsr = nc.dram_tensor("bsr", [1, 2], f32, kind="ExternalInput").ap()
    d_bsc = nc.dram_tensor("bsc", [2, 1], f32, kind="ExternalInput").ap()
    d_sel2 = nc.dram_tensor("sel2", [2, 256], f32, kind="ExternalInput").ap()
    d_out = nc.dram_tensor("outT", [D, TC], f32, kind="ExternalOutput").ap()

    with tile.TileContext(nc) as tc:
        import contextlib
        ctx = contextlib.ExitStack()
        with ctx:
            cpool = ctx.enter_context(tc.tile_pool(name="const", bufs=1))
            wpool = ctx.enter_context(tc.tile_pool(name="wstream", bufs=2))
            spool = ctx.enter_context(tc.tile_pool(name="work", bufs=2))
            pspool = ctx.enter_context(
                tc.tile_pool(name="ps", bufs=1, space="PSUM"))
            drpool = ctx.enter_context(
                tc.tile_pool(name="dram", bufs=1, space="DRAM"))

            # ---- prologue DMAs: base-GEMM inputs first (PE starts on
            # base(0) immediately; router waits for xT behind it) ----
            xTb_sb = cpool.tile([128, KT * TC], bf16, name="xTb_sb")
            XQ = KT * TC // 4

            def load_xtb(q):
                nc.sync.dma_start(out=xTb_sb[:, q * XQ:(q + 1) * XQ],
                                  in_=d_xTb[:, q * XQ:(q + 1) * XQ])
            load_xtb(0)

            # streamed weights: one DMA per h-tile
            def load_wug(i):
                t = wpool.tile([128, 2 * KT * 128], bf16, tag="wug", bufs=4,
                               name="wug_t")
                nc.sync.dma_start(out=t[:], in_=d_wug[i])
                return t

            def load_lora(i):
                t = wpool.tile([128, 3 * 128], bf16, tag="lora", bufs=W + 6,
                               name="lora_t")
                nc.sync.dma_start(out=t[:], in_=d_lora[i])
                return t

            WDQ = HT // 4 * 128        # quarter of wd's free columns (1408)

            def load_wd_q(di, q):
                t = wpool.tile([128, WDQ], bf16, tag="wd", bufs=26,
                               name="wd_t")
                nc.sync.dma_start(out=t[:], in_=d_wd[di][:, q * WDQ:
                                                         (q + 1) * WDQ])
                return t

            # tile 0's weights in pieces so base(0) starts ~2us in
            wug0 = wpool.tile([128, 2 * KT * 128], bf16, tag="wug", bufs=4,
                              name="wug0")
            UGH = KT * 128
            nc.sync.dma_start(out=wug0[:, 0:UGH // 2],
                              in_=d_wug[0][:, 0:UGH // 2])
            load_xtb(1)
            nc.sync.dma_start(out=wug0[:, UGH // 2:UGH],
                              in_=d_wug[0][:, UGH // 2:UGH])
            load_xtb(2)
            load_xtb(3)
            nc.sync.dma_start(out=wug0[:, UGH:2 * UGH],
                              in_=d_wug[0][:, UGH:2 * UGH])
            wug_t = {0: wug0}
            wug_t[1] = load_wug(1)
            wug_t[2] = load_wug(2)
            gwb_sb = cpool.tile([128, KT * E], bf16, name="gwb_sb")
            nc.sync.dma_start(out=gwb_sb[:], in_=d_gwb[:])
            gwr_sb = cpool.tile([128, KT * E], bf16, name="gwr_sb")
            nc.sync.dma_start(out=gwr_sb[:], in_=d_gwr[:])
            xr_sb = cpool.tile([128, KT * TC], bf16, name="xr_sb")
            nc.sync.dma_start(out=xr_sb[:], in_=d_xr[:])
            wug_t[3] = load_wug(3)
            A_sb = cpool.tile([128, KT * 2 * ER], bf16, name="A_sb")
            nc.sync.dma_start(out=A_sb[:], in_=d_A[:])
            eid_sb = cpool.tile([128, 1], f32, name="eid_sb")
            nc.sync.dma_start(out=eid_sb[:], in_=d_eid[:])
            i8m_sb = cpool.tile([128, E], f32, name="i8m_sb")
            nc.sync.dma_start(out=i8m_sb[:], in_=d_i8m[:])
            bsr_sb = cpool.tile([1, 2], f32, name="bsr_sb")
            nc.sync.dma_start(out=bsr_sb[:], in_=d_bsr[:])
            bsc_sb = cpool.tile([2, 1], f32, name="bsc_sb")
            nc.sync.dma_start(out=bsc_sb[:], in_=d_bsc[:])
            sel2_sb = cpool.tile([2, 256], f32, name="sel2_sb")
            nc.sync.dma_start(out=sel2_sb[:], in_=d_sel2[:])
            lora_t = {i: load_lora(i) for i in range(4)}
            dB_sb = cpool.tile([128, D], bf16, name="dB_sb")

            ident = cpool.tile([128, 128], f32, name="ident")
            make_identity(nc, ident)
            ones_col = cpool.tile([128, 1], f32, name="ones_col")
            nc.vector.memset(ones_col, 1.0)

            mixed = cpool.tile([128, HT * TC], bf16, name="mixed")
            ev_rows = cpool.tile([2, TC], f32, name="ev_rows")
            s_rows = cpool.tile([2, TC], f32, name="s_rows")
            crows = cpool.tile([2, TC], f32, name="crows")
            cb = cpool.tile([128, 2 * TC], bf16, name="cb")
            Mj = cpool.tile([128, 2 * TC], bf16, name="Mj")
            UA = cpool.tile([128, TC], bf16, name="UA")
            GA = cpool.tile([128, TC], bf16, name="GA")
            Ut = cpool.tile([128, 2 * TC], bf16, name="Ut")
            Gt = cpool.tile([128, 2 * TC], bf16, name="Gt")
            vt = cpool.tile([128, 2 * TC], bf16, name="vt")
            vts = cpool.tile([128, TC], bf16, name="vts")

            # ---- phase 1: router (f32), emitted in small stages between
            # warmup base tiles so no PE op waits long on DVE/collective.
            # Only cb (routing weights) depends on the AllReduce; masks Mj
            # and Ut/Gt do not, so the AR only gates the tail of lora(0).
            den_parts = cpool.tile([1, 4], f32, name="den_parts")
            tk = {}

            def emit_logits(tt):
                psL = pspool.tile([128, TC], f32, tag="ps_small", name="psL")
                # exact f32 logits from bf16 pairs: (xb+xr) @ (gb+gr)
                terms = []
                for k in range(KT):
                    c0 = k * TC + tt * 128
                    for xs in (xTb_sb, xr_sb):
                        for gs in (gwb_sb, gwr_sb):
                            terms.append((xs[:, c0:c0 + 128],
                                          gs[:, k * E:(k + 1) * E]))
                for n, (xs, gs) in enumerate(terms):
                    nc.tensor.matmul(psL[:, 0:E], xs, gs,
                                     start=(n == 0),
                                     stop=(n == len(terms) - 1))
                L = spool.tile([128, E], f32, tag="L", name="L")
                nc.vector.tensor_copy(L[:], psL[:, 0:E])
                mx1 = spool.tile([128, 1], f32, tag="mx1", name="mx1")
                nc.vector.tensor_reduce(mx1[:], L[:], mybir.AxisListType.X,
                                        AL.max)
                msk = spool.tile([128, E], f32, tag="msk", name="msk")
                nc.vector.tensor_scalar(msk[:], L[:], mx1[:], None,
                                        AL.is_equal)
                mi = spool.tile([128, E], f32, tag="mi", name="mi")
                nc.vector.tensor_tensor(mi[:], msk[:], i8m_sb[:], AL.mult)
                svals = spool.tile([128, 2], f32, tag="svals", name="svals")
                nc.vector.tensor_reduce(svals[:, 0:1], mi[:],
                                        mybir.AxisListType.X, AL.max)
                evals = spool.tile([128, 2], f32, tag="evals", name="evals")
                nc.scalar.activation(evals[:, 0:1], mx1[:], AF.Exp)
                # mask out slot-0 winner, find second max
                big = spool.tile([128, E], f32, tag="big", name="big")
                nc.vector.tensor_scalar(big[:], msk[:], 1e30, None, AL.mult)
                L2 = spool.tile([128, E], f32, tag="L2", name="L2")
                nc.vector.tensor_tensor(L2[:], L[:], big[:], AL.subtract)
                mx2 = spool.tile([128, 1], f32, tag="mx2", name="mx2")
                nc.vector.tensor_reduce(mx2[:], L2[:], mybir.AxisListType.X,
                                        AL.max)
                msk2 = spool.tile([128, E], f32, tag="msk2", name="msk2")
                nc.vector.tensor_scalar(msk2[:], L2[:], mx2[:], None,
                                        AL.is_equal)
                mi2 = spool.tile([128, E], f32, tag="mi2", name="mi2")
                nc.vector.tensor_tensor(mi2[:], msk2[:], i8m_sb[:], AL.mult)
                nc.vector.tensor_reduce(svals[:, 1:2], mi2[:],
                                        mybir.AxisListType.X, AL.max)
                nc.scalar.activation(evals[:, 1:2], mx2[:], AF.Exp)
                tk[tt] = (evals, svals)

            def emit_topk(tt):
                evals, svals = tk[tt]
                # per-tile partial denominators: [1,2] = ones.T @ evals
                psd = pspool.tile([1, 2], f32, tag="ps_small", name="psd")
                nc.tensor.matmul(psd[:], ones_col[:], evals[:],
                                 start=True, stop=True)
                nc.vector.tensor_copy(den_parts[:, tt * 2:(tt + 1) * 2],
                                      psd[:])
                # transpose evals/svals -> rows
                psT = pspool.tile([2, 128], f32, tag="ps_small", name="psT")
                nc.tensor.transpose(psT[:], evals[:], ident[:])
                nc.vector.tensor_copy(ev_rows[:, tt * 128:(tt + 1) * 128],
                                      psT[:])
                psT2 = pspool.tile([2, 128], f32, tag="ps_small", name="psT2")
                nc.tensor.transpose(psT2[:], svals[:], ident[:])
                nc.vector.tensor_copy(s_rows[:, tt * 128:(tt + 1) * 128],
                                      psT2[:])

            def emit_ar():
                # combine partials, AllReduce [2,8] (row b = batch)
                drow = cpool.tile([1, 2], f32, name="drow")
                nc.vector.tensor_tensor(drow[:], den_parts[:, 0:2],
                                        den_parts[:, 2:4], AL.add)
                ar_sb = cpool.tile([2, 8], f32, name="ar_sb")
                nc.vector.memset(ar_sb, 0.0)
                psAR = pspool.tile([2, 2], f32, tag="ps_small", name="psAR")
                nc.tensor.matmul(psAR[:], bsr_sb[:], drow[:], start=True,
                                 stop=True)
                nc.vector.tensor_copy(ar_sb[:, 0:2], psAR[:])
                ar_in = drpool.tile([2, 8], f32, name="ar_in")
                ar_out = drpool.tile([2, 8], f32, name="ar_out",
                                     addr_space="Shared")
                nc.gpsimd.dma_start(out=ar_in[:], in_=ar_sb[:])
                if SKIP_AR:
                    nc.gpsimd.dma_start(out=ar_out[:], in_=ar_in[:])
                else:
                    nc.gpsimd.collective_compute(
                        "AllReduce", AL.add,
                        replica_groups=[list(range(NCORES))],
                        ins=[ar_in.opt()], outs=[ar_out.opt()])
                den2 = cpool.tile([2, 8], f32, name="den2")
                nc.gpsimd.dma_start(out=den2[:], in_=ar_out[:])
                tk["den2"] = den2

            def emit_denrecv():
                # select this core's batch row -> [2(slots),1], reciprocal
                psDC = pspool.tile([2, 1], f32, tag="ps_small", name="psDC")
                nc.tensor.matmul(psDC[:], tk["den2"][:, 0:2], bsc_sb[:],
                                 start=True, stop=True)
                rcp = cpool.tile([2, 1], f32, name="rcp")
                nc.vector.reciprocal(rcp[:], psDC[:])
                # normalized routing weights as rows [2, TC]
                nc.vector.tensor_scalar(crows[:], ev_rows[:], rcp[:], None,
                                        AL.mult)

            def emit_mj():
                # masks from top-k indices (no AR dependency), then Ut/Gt
                for j in range(2):
                    psM = pspool.tile([128, TC], f32, tag="ps_small",
                                      name="psM")
                    nc.tensor.matmul(psM[:],
                                     sel2_sb[:, j * 128:(j + 1) * 128],
                                     s_rows[:], start=True, stop=True)
                    nc.vector.tensor_scalar(Mj[:, j * TC:(j + 1) * TC],
                                            psM[:], eid_sb[:], None,
                                            AL.is_equal)
                for j in range(2):
                    nc.vector.tensor_tensor(Ut[:, j * TC:(j + 1) * TC],
                                            UA[:],
                                            Mj[:, j * TC:(j + 1) * TC],
                                            AL.mult)
                    nc.vector.tensor_tensor(Gt[:, j * TC:(j + 1) * TC],
                                            GA[:],
                                            Mj[:, j * TC:(j + 1) * TC],
                                            AL.mult)

            def emit_cb():
                # routing weights broadcast along partitions (AR-dependent)
                for j in range(2):
                    psB = pspool.tile([128, TC], f32, tag="ps_small",
                                      name="psB")
                    nc.tensor.matmul(psB[:],
                                     sel2_sb[:, j * 128:(j + 1) * 128],
                                     crows[:], start=True, stop=True)
                    nc.vector.tensor_copy(cb[:, j * TC:(j + 1) * TC], psB[:])

            # ---- phase 3: stacked A-projections (bf16) ----
            def emit_aproj():
                psUA = pspool.tile([128, TC], f32, tag="psUG", bufs=2,
                                   name="psUA")
                for k in range(KT):
                    nc.tensor.matmul(psUA[:],
                                     A_sb[:, k * 2 * ER: k * 2 * ER + ER],
                                     xTb_sb[:, k * TC:(k + 1) * TC],
                                     start=(k == 0), stop=(k == KT - 1))
                nc.scalar.copy(UA[:], psUA[:])
                psGA = pspool.tile([128, TC], f32, tag="psUG", bufs=2,
                                   name="psGA")
                for k in range(KT):
                    nc.tensor.matmul(
                        psGA[:],
                        A_sb[:, k * 2 * ER + ER:(k + 1) * 2 * ER],
                        xTb_sb[:, k * TC:(k + 1) * TC],
                        start=(k == 0), stop=(k == KT - 1))
                nc.scalar.copy(GA[:], psGA[:])

            # ---- h-loop: software pipeline, base leads lora by W tiles,
            # then double-lora iterations shrink the lag to 3 (no tail drain)
            psV = pspool.tile([128, 2 * TC], f32, tag="psV", name="psV")
            U_ring = {}
            G_ring = {}
            pend = []                  # lora indices awaiting psV emission
            pend_t = {}                # i -> (dA slice, ch_pair)
            wd_pre = {}                # (di, q) -> tile
            wd_sched = []              # (iteration, di, q) prefetch slots
            for n in range(NPRE * 4):
                wd_sched.append((1 + n if n < 8 else 9 + 2 * (n - 8),
                                 n // 4, n % 4))
            wd_ptr = 0

            def base_tile(j):
                wt = wug_t.pop(j)
                psUG = pspool.tile([128, 2 * TC], f32, tag="psUG", bufs=2,
                                   name="psUG")
                for k in range(KT):
                    nc.tensor.matmul(psUG[:, 0:TC],
                                     wt[:, k * 128:(k + 1) * 128],
                                     xTb_sb[:, k * TC:(k + 1) * TC],
                                     start=(k == 0), stop=(k == KT - 1))
                for k in range(KT):
                    nc.tensor.matmul(psUG[:, TC:2 * TC],
                                     wt[:, (KT + k) * 128:(KT + k + 1) * 128],
                                     xTb_sb[:, k * TC:(k + 1) * TC],
                                     start=(k == 0), stop=(k == KT - 1))
                U_sb = spool.tile([128, TC], bf16, tag="U_sb", bufs=W + 2,
                                  name="U_sb")
                nc.scalar.copy(U_sb[:], psUG[:, 0:TC])
                G_sb = spool.tile([128, TC], bf16, tag="G_sb", bufs=W + 2,
                                  name="G_sb")
                nc.scalar.copy(G_sb[:], psUG[:, TC:2 * TC])
                U_ring[j] = U_sb
                G_ring[j] = G_sb

            def flush_psv(upto):
                # emit psV contractions for pending loras <= upto
                while pend and pend[0] <= upto:
                    l = pend.pop(0)
                    pv_dA, pv_ch = pend_t.pop(l)
                    nc.tensor.matmul(psV[:], pv_dA, pv_ch[:],
                                     start=(l == 0), stop=(l == HT - 1),
                                     skip_group_check=True)

            def lora_tile(i):
                lt = lora_t[i]
                psLOu = pspool.tile([128, 2 * TC], f32, tag="psLOu", bufs=2,
                                    name="psLOu")
                psLOg = pspool.tile([128, 2 * TC], f32, tag="psLOg", bufs=2,
                                    name="psLOg")
                nc.tensor.matmul(psLOu[:], lt[:, 0:128], Ut[:],
                                 start=True, stop=True)
                nc.tensor.matmul(psLOg[:], lt[:, 128:256], Gt[:],
                                 start=True, stop=True)
                ch_pair = spool.tile([128, 2 * TC], bf16, tag="chp", bufs=6,
                                     name="ch_pair")
                U_sb = U_ring.pop(i)
                G_sb = G_ring.pop(i)
                # slot 0: Activation copies PSUM->SBUF, DVE adds in bf16
                lu = spool.tile([128, TC], bf16, tag="lu", name="lu")
                nc.scalar.copy(lu[:], psLOu[:, 0:TC])
                lg = spool.tile([128, TC], bf16, tag="lg", name="lg")
                nc.scalar.copy(lg[:], psLOg[:, 0:TC])
                tu0 = spool.tile([128, TC], bf16, tag="tu0", name="tu0")
                nc.vector.tensor_tensor(tu0[:], U_sb[:], lu[:], AL.add)
                tg0 = spool.tile([128, TC], bf16, tag="tg0", name="tg0")
                nc.vector.tensor_tensor(tg0[:], G_sb[:], lg[:], AL.add)
                # slot 1: DVE reads PSUM directly (keeps Activation light)
                tu1 = spool.tile([128, TC], bf16, tag="tu1", name="tu1")
                nc.vector.tensor_tensor(tu1[:], U_sb[:], psLOu[:, TC:2 * TC],
                                        AL.add)
                tg1 = spool.tile([128, TC], bf16, tag="tg1", name="tg1")
                nc.vector.tensor_tensor(tg1[:], G_sb[:], psLOg[:, TC:2 * TC],
                                        AL.add)
                for jj, (tu, tg) in enumerate(((tu0, tg0), (tu1, tg1))):
                    su = spool.tile([128, TC], bf16, tag="su", name="su")
                    nc.scalar.activation(su[:], tu[:], AF.Silu)
                    hh = spool.tile([128, TC], bf16, tag="hh", name="hh")
                    nc.vector.tensor_tensor(hh[:], su[:], tg[:], AL.mult)
                    nc.vector.tensor_tensor(ch_pair[:, jj * TC:(jj + 1) * TC],
                                            hh[:],
                                            cb[:, jj * TC:(jj + 1) * TC],
                                            AL.mult)
                nc.vector.tensor_tensor(mixed[:, i * TC:(i + 1) * TC],
                                        ch_pair[:, 0:TC],
                                        ch_pair[:, TC:2 * TC], AL.add)
                pend.append(i)
                pend_t[i] = (lt[:, 256:384], ch_pair)

            # warmup: W base tiles ahead; router stages interleaved so no
            # PE-queue op ever waits long on a DVE/collective dependency
            for wi in range(W):
                if wi + 4 < HT and wi < W - 1:
                    wug_t[wi + 4] = load_wug(wi + 4)
                base_tile(wi)
                if wi == 2:
                    emit_logits(0)
                elif wi == 3:
                    emit_logits(1)
                elif wi == 4:
                    emit_aproj()
                elif wi == 5:
                    emit_topk(0)
                elif wi == 6:
                    emit_topk(1)
                elif wi == 7:
                    emit_ar()
                elif wi == 8:
                    emit_mj()
                elif wi == W - 2:
                    emit_denrecv()
                elif wi == W - 1:
                    emit_cb()

            li = 0                     # lora cursor
            bi = W                     # base cursor
            nl = 4                     # next lora to load
            it = 0                     # iteration counter (for wd prefetch)
            while li < HT:
                if bi < HT:
                    if bi + 1 < HT and bi + 1 not in wug_t:
                        wug_t[bi + 1] = load_wug(bi + 1)
                    base_tile(bi)
                    bi += 1
                while nl < HT and nl < li + 6:
                    lora_t[nl] = load_lora(nl)
                    nl += 1
                if it == 0:
                    nc.sync.dma_start(out=dB_sb[:], in_=d_dB[:])
                while wd_ptr < len(wd_sched) and wd_sched[wd_ptr][0] <= it:
                    _, pdi, pq = wd_sched[wd_ptr]
                    wd_pre[(pdi, pq)] = load_wd_q(pdi, pq)
                    wd_ptr += 1
                flush_psv(li - 2)
                lora_tile(li)
                li += 1
                if bi - li > 3 and li < HT:
                    flush_psv(li - 2)
                    lora_tile(li)
                    li += 1
                it += 1

            flush_psv(HT - 1)
            # masked v, then fold both slots (down_B is linear)
            nc.vector.tensor_tensor(vt[:], psV[:], Mj[:], AL.mult)
            nc.vector.tensor_tensor(vts[:], vt[:, 0:TC], vt[:, TC:2 * TC],
                                    AL.add)

            # ---- down GEMM + LoRA-down, wd streamed with 2-tile lead ----
            def wd_quarter(di, q):
                t = wd_pre.pop((di, q), None)
                return t if t is not None else load_wd_q(di, q)

            wd_cur = {(di, q): wd_quarter(di, q)
                      for di in range(NPRE + 2) for q in range(4)
                      if di < DT}
            for di in range(DT):
                if di + 2 < DT and (di + 2, 0) not in wd_cur:
                    for q in range(4):
                        wd_cur[(di + 2, q)] = wd_quarter(di + 2, q)
                wd_h = [wd_cur.pop((di, q)) for q in range(4)]
                psO = pspool.tile([128, TC], f32, tag="psUG", bufs=2,
                                  name="psO")
                for hk in range(HT):
                    w = wd_h[hk // 11][:, (hk % 11) * 128:(hk % 11 + 1) * 128]
                    nc.tensor.matmul(psO[:], w,
                                     mixed[:, hk * TC:(hk + 1) * TC],
                                     start=(hk == 0), stop=False,
                                     skip_group_check=True)
                nc.tensor.matmul(psO[:], dB_sb[:, di * 128:(di + 1) * 128],
                                 vts[:], start=False, stop=True,
                                 skip_group_check=True)
                o_sb = spool.tile([128, TC], f32, tag="o_sb", name="o_sb")
                nc.scalar.copy(o_sb[:], psO[:])
                nc.sync.dma_start(out=d_out[di * 128:(di + 1) * 128, :],
                                  in_=o_sb[:])

    nc.compile()
    return nc


def _prep_shared(inputs):
    """Host-side layout prep of weight tensors (shared across cores)."""
    import ml_dtypes
    bf16 = np.dtype(ml_dtypes.bfloat16)
    f32 = np.float32

    def c(a, dt):
        return np.ascontiguousarray(a.astype(dt, copy=False))

    w_up, w_gate, w_down = inputs["w_up"], inputs["w_gate"], inputs["w_down"]
    # wu[i][kp, k*128+h] = w_up[i*128+h, k*128+kp] (lhsT per k-tile)
    wu = (w_up.reshape(HT, 128, KT, 128).transpose(0, 3, 2, 1)
          .reshape(HT, 128, KT * 128))
    wg = (w_gate.reshape(HT, 128, KT, 128).transpose(0, 3, 2, 1)
          .reshape(HT, 128, KT * 128))
    wug = c(np.concatenate([wu, wg], axis=2), bf16)
    wd = c(w_down.reshape(DT, 128, HT, 128).transpose(0, 3, 2, 1)
           .reshape(DT, 128, HT * 128), bf16)

    A_stack = np.concatenate([
        inputs["up_A"].reshape(ER, D),
        inputs["gate_A"].reshape(ER, D)], axis=0)          # [2*ER, D]
    # Ah[p, k*2ER + m] = A_stack[m, k*128+p]
    Ah = c(A_stack.reshape(2 * ER, KT, 128).transpose(2, 1, 0)
           .reshape(128, KT * 2 * ER), bf16)

    up_B_all = (inputs["up_B"].transpose(0, 2, 1).reshape(ER, H)
                * ALPHA).astype(f32)
    gate_B_all = (inputs["gate_B"].transpose(0, 2, 1).reshape(ER, H)
                  * ALPHA).astype(f32)
    uB = up_B_all.reshape(ER, HT, 128).transpose(1, 0, 2)   # [HT, er, h]
    gB = gate_B_all.reshape(ER, HT, 128).transpose(1, 0, 2)
    down_A_all = inputs["down_A"].reshape(ER, H).astype(f32)
    dA = down_A_all.T.reshape(HT, 128, ER)                  # [HT, h, er]
    lorah = c(np.concatenate([uB, gB, dA], axis=2), bf16)   # [HT, 128, 384]

    down_B_all = (inputs["down_B"].transpose(0, 2, 1).reshape(ER, D)
                  * ALPHA).astype(f32)
    dB = c(down_B_all, bf16)

    gate_wT = inputs["gate_w"].T.astype(f32)               # [D, E]
    gw = np.ascontiguousarray(
        gate_wT.reshape(KT, 128, E).transpose(1, 0, 2)
        .reshape(128, KT * E)).astype(f32)
    gwb = gw.astype(bf16)
    gwr = (gw - gwb.astype(f32)).astype(bf16)

    eid = (8.0 - (np.arange(128) // R)).astype(f32).reshape(128, 1)
    i8m = np.tile((8.0 - np.arange(E)).astype(f32), (128, 1))
    sel2 = np.zeros((2, 256), f32)
    sel2[0, 0:128] = 1.0
    sel2[1, 128:256] = 1.0

    return dict(wug=wug, wd=wd, Ah=Ah, lorah=lorah, dB=dB,
                gwb=gwb, gwr=gwr, eid=eid, i8m=i8m, sel2=sel2)


def kernel(**inputs):
    from concourse.bass_utils import run_bass_kernel_spmd
    import ml_dtypes
    bf16 = np.dtype(ml_dtypes.bfloat16)

    inputs = {k: np.asarray(v) for k, v in inputs.items()}
    if "nc" not in _cache:
        _cache["nc"] = _build()
    nc = _cache["nc"]

    shared = _prep_shared(inputs)
    x = inputs["x"].astype(np.float32)
    xt = x.reshape(T, D)

    in_maps = []
    for cix in range(NCORES):
        xc = xt[cix * TC:(cix + 1) * TC]                   # [TC, D]
        # xTh[p, k*TC+t] = xc[t, k*128+p]
        xTh = np.ascontiguousarray(
            xc.T.reshape(KT, 128, TC).transpose(1, 0, 2)
            .reshape(128, KT * TC))
        b = (cix * TC) // S
        bsr = np.zeros((1, 2), np.float32); bsr[0, b] = 1.0
        bsc = np.zeros((2, 1), np.float32); bsc[b, 0] = 1.0
        xb = xTh.astype(bf16)
        xr = (xTh - xb.astype(np.float32)).astype(bf16)
        m = dict(shared)
        m["xTb"] = np.ascontiguousarray(xb)
        m["xr"] = np.ascontiguousarray(xr)
        m["bsr"] = bsr
        m["bsc"] = bsc
        in_maps.append(m)

    res = run_bass_kernel_spmd(nc, in_maps, list(range(NCORES)))
    out = np.empty((T, D), np.float32)
    for cix in range(NCORES):
        out[cix * TC:(cix + 1) * TC, :] = res.results[cix]["outT"].T
    return out.reshape(B, S, D)
